# revision 12
# baseline (speedup 1.0000x reference)
"""Trainium2 Bass kernel for column-softmax attention.

reference semantics:
    scores = einsum('bqd,bkd->bqk', q, k) / sqrt(128)   # [B, Nq, Nk]
    attn   = softmax(scores, axis=1)                     # over the QUERY axis
    out    = einsum('bqk,bkd->bqd', attn, v)             # [B, Nq, D]

Because the softmax is over q, each key column k normalizes independently:
    out[q, d] = sum_k E[k, q] * r[k] * v[k, d],  E = exp(scores.T), r = 1/sum_q E[k, q]

Sharding: 8 cores = 4 batches x 2 key-halves.  Each core computes the partial
sum over its 2048 keys; the host adds the two partials per batch.

On-chip layout: the host pre-transposes Q and K to [D, N] (contraction dim on
partitions) and the kernel emits out.T [D, Nq]; the host transposes back.  The
softmax denominator is folded into V row-scaling so the normalize step touches
only 128x128 tiles per key tile.

Phase A (per key tile): scores matmul (fp16 in, fp32 psum) -> exp on ScalarE
(with fused row-sum accumulation) -> E resident in SBUF as fp16.
Phase B: out.T accumulated over all 16 key tiles directly in PSUM.
"""

import numpy as np

import concourse.bass as bass
import concourse.mybir as mybir
import concourse.tile as tile
from concourse.bass_utils import run_bass_kernel_spmd

B, N, D = 4, 4096, 128
P = 128
NK = 2048                 # keys per core (half of 4096)
KT_TILES = NK // P        # 16 key tiles of 128
SCALE = 1.0 / np.sqrt(128.0)

F32 = mybir.dt.float32
F16 = mybir.dt.float16


def emit_body(nc, tc, pools, aps):
    big, epool, small, spsum = pools
    qt_d, kt_d, v_d, out_d = aps

    qT = big.tile([P, N], F16, tag="qT")            # [d, q]
    kT = big.tile([P, NK], F16, tag="kT")           # [d, k]
    vsb = big.tile([P, KT_TILES, D], F32, tag="v")  # [k_in_tile, k_tile, d]
    oacc = big.tile([P, N], F32, tag="oacc")        # [d, q]

    nc.sync.dma_start(qT[:], qt_d[:])
    nc.sync.dma_start(kT[:], kt_d[:])
    nc.sync.dma_start(vsb[:], v_d.rearrange("(t p) d -> p t d", p=P))

    # Warm-up matmul: first real matmul then carries at most one sync wait.
    Swarm = spsum.tile([P, 2048], F32, tag="S")
    nc.tensor.matmul(
        Swarm[0:1, 0:1], lhsT=kT[:, 0:1], rhs=qT[:, 0:1], start=True, stop=True
    )

    e_tiles = []
    v_tiles = []
    # Phase A: per key tile, scores + exp (row-sum fused) + scaled V.
    for kt in range(KT_TILES):
        E = epool.tile([P, N], F16, tag=f"E{kt}")   # [k, q] = exp(scores.T)
        rs = small.tile([P, 2], F32, tag=f"rs{kt}")
        for h in range(2):
            S = spsum.tile([P, 2048], F32, tag="S")
            for u in range(4):
                nc.tensor.matmul(
                    S[:, u * 512 : (u + 1) * 512],
                    lhsT=kT[:, kt * P : (kt + 1) * P],
                    rhs=qT[:, h * 2048 + u * 512 : h * 2048 + (u + 1) * 512],
                    start=True,
                    stop=True,
                )
            nc.scalar.activation(
                out=E[:, h * 2048 : (h + 1) * 2048],
                in_=S[:],
                func=mybir.ActivationFunctionType.Exp,
                scale=float(SCALE),
                accum_out=rs[:, h : h + 1],
            )
        rsum = small.tile([P, 1], F32, tag="rsum")
        nc.vector.reduce_sum(out=rsum[:], in_=rs[:], axis=mybir.AxisListType.X)
        recip = small.tile([P, 1], F32, tag="recip")
        nc.vector.reciprocal(recip[:], rsum[:])
        vsc = small.tile([P, D], F16, tag=f"vsc{kt}")  # [k, d] * r[k]
        nc.vector.tensor_scalar_mul(vsc[:], vsb[:, kt, :], recip[:])
        e_tiles.append(E)
        v_tiles.append(vsc)

    # Phase B: out.T accumulated over all key tiles in PSUM.
    # O tiles share the S slots (tag "S") so PSUM stays within 8 banks.
    for c in range(2):
        O = spsum.tile([P, 2048], F32, tag="S")
        for u in range(4):
            dst = O[:, u * 512 : (u + 1) * 512]
            for kt in range(KT_TILES):
                nc.tensor.matmul(
                    dst,
                    lhsT=v_tiles[kt][:],
                    rhs=e_tiles[kt][:, c * 2048 + u * 512 : c * 2048 + (u + 1) * 512],
                    start=(kt == 0),
                    stop=(kt == KT_TILES - 1),
                )
        nc.vector.tensor_copy(out=oacc[:, c * 2048 : (c + 1) * 2048], in_=O[:])

    nc.sync.dma_start(out_d[:], oacc[:])


def build_bass(repeat=1):
    nc = bass.Bass("TRN2", target_bir_lowering=False, debug=False)
    qt_d = nc.dram_tensor("qt", [P, N], F16, kind="ExternalInput").ap()
    kt_d = nc.dram_tensor("kt", [P, NK], F16, kind="ExternalInput").ap()
    v_d = nc.dram_tensor("v", [NK, D], F32, kind="ExternalInput").ap()
    out_d = nc.dram_tensor("out_t", [P, N], F32, kind="ExternalOutput").ap()

    with tile.TileContext(nc) as tc:
        with (
            tc.tile_pool(name="big", bufs=1) as big,
            tc.tile_pool(name="epool", bufs=1) as epool,
            tc.tile_pool(name="small", bufs=2) as small,
            tc.tile_pool(name="spsum", bufs=2, space="PSUM") as spsum,
        ):
            for _ in range(repeat):
                emit_body(
                    nc,
                    tc,
                    (big, epool, small, spsum),
                    (qt_d, kt_d, v_d, out_d),
                )
    return nc


def legalize_waits(nc, max_waits=1):
    """Hoist excess semaphore waits into standalone EventSemaphore ops.

    The walrus codegen for several engine instruction structs accepts only a
    single sync-wait command; Tile sometimes emits more.  Executing the extra
    waits in a preceding same-engine EventSemaphore is semantically identical
    (the engine runs its stream in order).
    """
    for fn in nc.m.functions:
        for blk in fn.blocks:
            out = []
            for inst in blk.instructions:
                si = inst.sync_info
                if (
                    si is not None
                    and si.on_wait
                    and len(si.on_wait) > max_waits
                    and inst.opcode != "EventSemaphore"
                ):
                    waits = list(si.on_wait)
                    extra, keep = waits[:-max_waits], waits[-max_waits:]
                    for n, w in enumerate(extra):
                        out.append(
                            mybir.InstEventSemaphore(
                                name=f"{inst.name}_prewait{n}",
                                engine=inst.engine,
                                ins=[],
                                outs=[],
                                sync_info=mybir.SyncInfo(on_wait=[w], on_update=[]),
                            )
                        )
                    si.on_wait = keep
                out.append(inst)
            blk.instructions = out
    return nc


_NC_CACHE = {}


def _get_nc(repeat=1):
    key = ("nc", repeat)
    if key not in _NC_CACHE:
        _NC_CACHE[key] = legalize_waits(build_bass(repeat))
    return _NC_CACHE[key]


def kernel(q, k, v):
    q = np.asarray(q, dtype=np.float32)
    k = np.asarray(k, dtype=np.float32)
    v = np.asarray(v, dtype=np.float32)

    in_maps = []
    for c in range(8):
        b, h = c // 2, c % 2
        in_maps.append(
            {
                "qt": np.ascontiguousarray(q[b].T).astype(np.float16),
                "kt": np.ascontiguousarray(k[b, h * NK : (h + 1) * NK].T).astype(np.float16),
                "v": np.ascontiguousarray(v[b, h * NK : (h + 1) * NK]),
            }
        )

    nc = _get_nc()
    res = run_bass_kernel_spmd(nc, in_maps, list(range(8))).results

    out = np.empty((B, N, D), dtype=np.float32)
    for b in range(B):
        out[b] = (res[2 * b]["out_t"] + res[2 * b + 1]["out_t"]).T
    return out


# revision 20
# speedup vs baseline: 1.3266x; 1.3266x over previous
"""Trainium2 Bass kernel for column-softmax attention.

reference semantics:
    scores = einsum('bqd,bkd->bqk', q, k) / sqrt(128)   # [B, Nq, Nk]
    attn   = softmax(scores, axis=1)                     # over the QUERY axis
    out    = einsum('bqk,bkd->bqd', attn, v)             # [B, Nq, D]

Because the softmax is over q, each key column k normalizes independently:
    out[q, d] = sum_k E[k, q] * r[k] * v[k, d],  E = exp(scores.T), r = 1/sum_q E[k, q]

Sharding: 8 cores = 4 batches x 2 key-halves.  Each core computes the partial
sum over its 2048 keys; the host adds the two partials per batch.

On-chip layout: the host pre-transposes Q and K to [D, N] (contraction dim on
partitions) and the kernel emits out.T [D, Nq]; the host transposes back.  The
softmax denominator is folded into V row-scaling so the normalize step touches
only 128x128 tiles per key tile.

Phase A (per key tile): scores matmul (fp16 in, fp32 psum) -> exp on ScalarE
(with fused row-sum accumulation) -> E resident in SBUF as fp16.
Phase B: out.T accumulated over all 16 key tiles directly in PSUM.
"""

import numpy as np

import concourse.bass as bass
import concourse.mybir as mybir
import concourse.tile as tile
from concourse.bass_utils import run_bass_kernel_spmd

B, N, D = 4, 4096, 128
P = 128
NK = 2048                 # keys per core (half of 4096)
KT_TILES = NK // P        # 16 key tiles of 128
SCALE = 1.0 / np.sqrt(128.0)

F32 = mybir.dt.float32
F16 = mybir.dt.float16


def emit_body(nc, tc, pools, aps, skip_act=False, skip_phaseb=False, skip_gemm1=False):
    big, epool, small, spsum, opsum = pools
    qt_d, kt_d, v_d, out_d = aps

    qT = big.tile([P, N], F16, tag="qT")            # [d, q]
    kT = big.tile([P, NK], F16, tag="kT")           # [d, k]
    vsb = big.tile([P, KT_TILES, D], F16, tag="v")  # [k_in_tile, k_tile, d]
    oacc = big.tile([P, N], F32, tag="oacc")        # [d, q]

    for _qc in range(4):
        nc.sync.dma_start(
            qT[:, _qc * 1024 : (_qc + 1) * 1024], qt_d[:, _qc * 1024 : (_qc + 1) * 1024]
        )
    nc.sync.dma_start(kT[:], kt_d[:])
    nc.sync.dma_start(vsb[:], v_d.rearrange("(t p) d -> p t d", p=P))

    # Warm-up matmul: first real matmul then carries at most one sync wait.
    Swarm = spsum.tile([P, 1024], F32, tag="S")
    nc.tensor.matmul(
        Swarm[0:1, 0:1], lhsT=kT[:, 0:1], rhs=qT[:, 0:1], start=True, stop=True
    )

    e_tiles = []
    v_tiles = []
    # Output accumulators for query half A (cols 0..2047) are built up during
    # phase A so most of the second GEMM hides under the exp (ScalarE) span.
    oa_tiles = []
    if not skip_phaseb:
        for _oc in range(2):
            O_a = opsum.tile([P, 1024], F32, tag="O")
            oa_tiles.append(O_a)

    # Phase A: per key tile, scores + exp (row-sum fused) + scaled V,
    # then this key tile's contribution to out.T for query half A.
    for kt in range(KT_TILES):
        E = epool.tile([P, N], F16, tag=f"E{kt}")   # [k, q] = exp(scores.T)
        rs = small.tile([P, 4], F32, tag=f"rs{kt}")
        for h in range(0 if skip_gemm1 else 4):
            S = spsum.tile([P, 1024], F32, tag="S")
            for u in range(2):
                nc.tensor.matmul(
                    S[:, u * 512 : (u + 1) * 512],
                    lhsT=kT[:, kt * P : (kt + 1) * P],
                    rhs=qT[:, h * 1024 + u * 512 : h * 1024 + (u + 1) * 512],
                    start=True,
                    stop=True,
                )
            if not skip_act:
                nc.scalar.activation(
                    out=E[:, h * 1024 : (h + 1) * 1024],
                    in_=S[:],
                    func=mybir.ActivationFunctionType.Exp,
                    scale=float(SCALE),
                    accum_out=rs[:, h : h + 1],
                )
        rsum = small.tile([P, 1], F32, tag="rsum")
        recip = small.tile([P, 1], F32, tag="recip")
        vsc = small.tile([P, D], F16, tag=f"vsc{kt}")  # [k, d] * r[k]
        if not skip_act:
            nc.vector.reduce_sum(out=rsum[:], in_=rs[:], axis=mybir.AxisListType.X)
            nc.vector.reciprocal(recip[:], rsum[:])
            nc.vector.tensor_scalar_mul(vsc[:], vsb[:, kt, :], recip[:])
        elif not skip_phaseb:
            nc.sync.dma_start(E[:], qt_d[:])
            nc.vector.tensor_copy(out=vsc[:], in_=vsb[:, kt, :])
        e_tiles.append(E)
        v_tiles.append(vsc)

        if not skip_phaseb:
            for oc in range(2):
                for u in range(2):
                    nc.tensor.matmul(
                        oa_tiles[oc][:, u * 512 : (u + 1) * 512],
                        lhsT=vsc[:],
                        rhs=E[:, oc * 1024 + u * 512 : oc * 1024 + (u + 1) * 512],
                        start=(kt == 0),
                        stop=(kt == KT_TILES - 1),
                    )

    # Phase B: flush half A, then accumulate query half B (cols 2048..4095).
    if not skip_phaseb:
        for oc in range(2):
            nc.vector.tensor_copy(
                out=oacc[:, oc * 1024 : (oc + 1) * 1024], in_=oa_tiles[oc][:]
            )
            nc.sync.dma_start(
                out_d[:, oc * 1024 : (oc + 1) * 1024],
                oacc[:, oc * 1024 : (oc + 1) * 1024],
            )
        ob_tiles = []
        for _oc in range(2):
            O_b = opsum.tile([P, 1024], F32, tag="O")
            ob_tiles.append(O_b)
        for kt in range(KT_TILES):
            for oc in range(2):
                for u in range(2):
                    nc.tensor.matmul(
                        ob_tiles[oc][:, u * 512 : (u + 1) * 512],
                        lhsT=v_tiles[kt][:],
                        rhs=e_tiles[kt][:, 2048 + oc * 1024 + u * 512 : 2048 + oc * 1024 + (u + 1) * 512],
                        start=(kt == 0),
                        stop=(kt == KT_TILES - 1),
                    )
        for oc in range(2):
            nc.vector.tensor_copy(
                out=oacc[:, 2048 + oc * 1024 : 2048 + (oc + 1) * 1024],
                in_=ob_tiles[oc][:],
            )
            nc.sync.dma_start(
                out_d[:, 2048 + oc * 1024 : 2048 + (oc + 1) * 1024],
                oacc[:, 2048 + oc * 1024 : 2048 + (oc + 1) * 1024],
            )
    else:
        nc.gpsimd.memset(oacc[:], 0.0)
        nc.sync.dma_start(out_d[:], oacc[:])


def build_bass(repeat=1, skip_act=False, skip_phaseb=False, skip_gemm1=False, loop=False):
    nc = bass.Bass("TRN2", target_bir_lowering=False, debug=False)
    qt_d = nc.dram_tensor("qt", [P, N], F16, kind="ExternalInput").ap()
    kt_d = nc.dram_tensor("kt", [P, NK], F16, kind="ExternalInput").ap()
    v_d = nc.dram_tensor("v", [NK, D], F16, kind="ExternalInput").ap()
    out_d = nc.dram_tensor("out_t", [P, N], F32, kind="ExternalOutput").ap()

    with tile.TileContext(nc) as tc:
        with (
            tc.tile_pool(name="big", bufs=1) as big,
            tc.tile_pool(name="epool", bufs=1) as epool,
            tc.tile_pool(name="small", bufs=2) as small,
            tc.tile_pool(name="spsum", bufs=2, space="PSUM") as spsum,
            tc.tile_pool(name="opsum", bufs=2, space="PSUM") as opsum,
        ):
            def body():
                emit_body(
                    nc,
                    tc,
                    (big, epool, small, spsum, opsum),
                    (qt_d, kt_d, v_d, out_d),
                    skip_act=skip_act,
                    skip_phaseb=skip_phaseb,
                    skip_gemm1=skip_gemm1,
                )

            if loop and repeat > 1:
                with tc.For_i(0, repeat, 1):
                    body()
            else:
                for _ in range(repeat):
                    body()
    return nc


def legalize_waits(nc, max_waits=1):
    """Hoist excess semaphore waits into standalone EventSemaphore ops.

    The walrus codegen for several engine instruction structs accepts only a
    single sync-wait command; Tile sometimes emits more.  Executing the extra
    waits in a preceding same-engine EventSemaphore is semantically identical
    (the engine runs its stream in order).
    """
    for fn in nc.m.functions:
        for blk in fn.blocks:
            out = []
            for inst in blk.instructions:
                si = inst.sync_info
                if (
                    si is not None
                    and si.on_wait
                    and len(si.on_wait) > max_waits
                    and inst.opcode != "EventSemaphore"
                ):
                    waits = list(si.on_wait)
                    extra, keep = waits[:-max_waits], waits[-max_waits:]
                    for n, w in enumerate(extra):
                        out.append(
                            mybir.InstEventSemaphore(
                                name=f"{inst.name}_prewait{n}",
                                engine=inst.engine,
                                ins=[],
                                outs=[],
                                sync_info=mybir.SyncInfo(on_wait=[w], on_update=[]),
                            )
                        )
                    si.on_wait = keep
                out.append(inst)
            blk.instructions = out
    return nc


_NC_CACHE = {}


def _get_nc(repeat=1, **kw):
    key = ("nc", repeat, tuple(sorted(kw.items())))
    if key not in _NC_CACHE:
        _NC_CACHE[key] = legalize_waits(build_bass(repeat, **kw))
    return _NC_CACHE[key]


def kernel(q, k, v):
    q = np.asarray(q, dtype=np.float32)
    k = np.asarray(k, dtype=np.float32)
    v = np.asarray(v, dtype=np.float32)

    in_maps = []
    for c in range(8):
        b, h = c // 2, c % 2
        in_maps.append(
            {
                "qt": np.ascontiguousarray(q[b].T).astype(np.float16),
                "kt": np.ascontiguousarray(k[b, h * NK : (h + 1) * NK].T).astype(np.float16),
                "v": np.ascontiguousarray(v[b, h * NK : (h + 1) * NK]).astype(np.float16),
            }
        )

    nc = _get_nc()
    res = run_bass_kernel_spmd(nc, in_maps, list(range(8))).results

    out = np.empty((B, N, D), dtype=np.float32)
    for b in range(B):
        out[b] = (res[2 * b]["out_t"] + res[2 * b + 1]["out_t"]).T
    return out
